# revision 1
# baseline (speedup 1.0000x reference)
"""BitLinear (ternary-quantized linear) Trainium2 kernel.

Computes: out = x @ dequant(weight).T where dequant is per-group(128)
AbsMean ternary quantization (w_q in {-1,0,+1} times per-group scale).

Strategy (8 NeuronCores, column-parallel / tensor-parallel):
  - weight [O=11008, K=4096] is sharded by rows across 8 cores (1376 each).
  - x [B,S,K] -> [T=8192, K] is replicated to every core, pre-transposed on
    host to [K, T] so the contraction dim lands on SBUF partitions.
  - Each core dequantizes its weight shard on-chip (no division needed:
    round(w/s) clipped to [-1,1] == (w > s/2) - (w < -s/2)), applies the
    per-group scale, transposes to [k, o] via the PE, and keeps the whole
    bf16 effective weight resident in SBUF.  x streams through in t-tiles
    of 128, accumulating in PSUM over the 32 k-groups.
  - Per-core output [T, 1376] (t-major); host concatenates along O.
"""

import os

import numpy as np

import concourse.bass as bass
import concourse.mybir as mybir
import concourse.tile as tile
from concourse import bacc
from concourse.bass_utils import run_bass_kernel_spmd
from concourse.masks import make_identity

P = 128
GROUP = 128
EPS = 1e-8

# Full problem shapes (hardcoded; harness calls kernel() with these).
FULL_B, FULL_S, FULL_K, FULL_O = 4, 2048, 4096, 11008
N_CORES = 8

LAST_RESULT = None  # BassKernelResults of the most recent run (for test.py)


def build_program(K, T, O_SHARD, mm_dt=mybir.dt.bfloat16):
    """One SPMD program, identical on every core (data differs per core).

    DRAM tensors:
      xt  [K, T] f32  ExternalInput   (x transposed, replicated)
      w   [O_SHARD, K] f32 ExternalInput (weight shard, natural layout)
      out [T, O_SHARD] f32 ExternalOutput
    """
    assert K % GROUP == 0 and T % P == 0
    KO = K // GROUP
    n_ttiles = T // P
    o_tiles = [(o0, min(P, O_SHARD - o0)) for o0 in range(0, O_SHARD, P)]
    OC = 512
    o_chunks = [(c0, min(OC, O_SHARD - c0)) for c0 in range(0, O_SHARD, OC)]

    nc = bacc.Bacc("TRN2", target_bir_lowering=False, debug=False)
    xt = nc.dram_tensor("xt", [K, T], mybir.dt.float32, kind="ExternalInput").ap()
    w = nc.dram_tensor(
        "w", [O_SHARD, K], mybir.dt.float32, kind="ExternalInput"
    ).ap()
    out = nc.dram_tensor(
        "out", [T, O_SHARD], mybir.dt.float32, kind="ExternalOutput"
    ).ap()

    with tile.TileContext(nc) as tc:
        with (
            tc.tile_pool(name="wres", bufs=1) as wres,
            tc.tile_pool(name="const", bufs=1) as constp,
        ):
            # Resident dequantized transposed weight: [k within group, group, o]
            wbt = wres.tile([P, KO, O_SHARD], mm_dt)
            ident = constp.tile([P, P], mm_dt)
            make_identity(nc, ident)

            # ---------------- Phase 1: dequant + transpose ----------------
            with (
                tc.tile_pool(name="ph1f32", bufs=2) as ph1f32,
                tc.tile_pool(name="ph1b", bufs=2) as ph1b,
                tc.tile_pool(name="tiny", bufs=2) as tiny,
                tc.tile_pool(name="ps_tp", bufs=4, space="PSUM") as ps_tp,
            ):
                for o0, osz in o_tiles:
                    wt = ph1f32.tile([P, KO, GROUP], mybir.dt.float32, tag="wt")
                    nc.sync.dma_start(
                        wt[:osz],
                        w[o0 : o0 + osz].rearrange("o (ko k) -> o ko k", k=GROUP),
                    )
                    sums = tiny.tile([P, KO], mybir.dt.float32, tag="sums")
                    nc.vector.tensor_reduce(
                        sums[:osz],
                        wt[:osz],
                        axis=mybir.AxisListType.X,
                        op=mybir.AluOpType.add,
                        apply_absolute_value=True,
                    )
                    # s = max(mean, EPS); thresholds +-s/2
                    s = tiny.tile([P, KO], mybir.dt.float32, tag="s")
                    nc.vector.tensor_scalar(
                        s[:osz],
                        sums[:osz],
                        1.0 / GROUP,
                        EPS,
                        mybir.AluOpType.mult,
                        mybir.AluOpType.max,
                    )
                    tpos = tiny.tile([P, KO], mybir.dt.float32, tag="tpos")
                    nc.vector.tensor_scalar_mul(tpos[:osz], s[:osz], 0.5)
                    tneg = tiny.tile([P, KO], mybir.dt.float32, tag="tneg")
                    nc.vector.tensor_scalar_mul(tneg[:osz], s[:osz], -0.5)

                    a = ph1b.tile([P, KO, GROUP], mm_dt, tag="a")
                    b = ph1b.tile([P, KO, GROUP], mm_dt, tag="b")
                    nc.vector.tensor_tensor(
                        a[:osz],
                        wt[:osz],
                        tpos[:osz, :, None].to_broadcast((osz, KO, GROUP)),
                        mybir.AluOpType.is_gt,
                    )
                    nc.vector.tensor_tensor(
                        b[:osz],
                        wt[:osz],
                        tneg[:osz, :, None].to_broadcast((osz, KO, GROUP)),
                        mybir.AluOpType.is_lt,
                    )
                    # q = a - b in {-1,0,1}; then wb = q * s (exact in bf16:
                    # +-s rounds once, products of {-1,0,1} are exact)
                    nc.vector.tensor_tensor(
                        a[:osz], a[:osz], b[:osz], mybir.AluOpType.subtract
                    )
                    nc.vector.tensor_tensor(
                        b[:osz],
                        a[:osz],
                        s[:osz, :, None].to_broadcast((osz, KO, GROUP)),
                        mybir.AluOpType.mult,
                    )
                    for ko in range(KO):
                        ps = ps_tp.tile([P, P], mm_dt, tag="tp")
                        nc.tensor.transpose(
                            ps[:, :osz], b[:osz, ko, :], ident[:osz, :osz]
                        )
                        nc.vector.tensor_copy(
                            wbt[:, ko, o0 : o0 + osz], ps[:, :osz]
                        )

            # ---------------- Phase 2: matmul ----------------
            with (
                tc.tile_pool(name="xf32", bufs=2) as xf32,
                tc.tile_pool(name="xb16", bufs=2) as xb16,
                tc.tile_pool(name="outp", bufs=2) as outp,
                tc.tile_pool(name="ps_mm", bufs=2, space="PSUM") as ps_mm,
            ):
                xt_r = xt.rearrange("(ko p) t -> p ko t", p=P)
                for tt in range(n_ttiles):
                    t0 = tt * P
                    xf = xf32.tile([P, KO, P], mybir.dt.float32, tag="xf")
                    nc.sync.dma_start(xf, xt_r[:, :, t0 : t0 + P])
                    xb = xb16.tile([P, KO, P], mm_dt, tag="xb")
                    nc.scalar.copy(xb, xf)

                    ot = outp.tile([P, O_SHARD], mybir.dt.float32, tag="ot")
                    pss = []
                    for ci, (c0, csz) in enumerate(o_chunks):
                        ps = ps_mm.tile([P, OC], mybir.dt.float32, tag=f"mm{ci}")
                        pss.append(ps[:, :csz])
                    for ko in range(KO):
                        for ci, (c0, csz) in enumerate(o_chunks):
                            nc.tensor.matmul(
                                pss[ci],
                                lhsT=xb[:, ko, :],
                                rhs=wbt[:, ko, c0 : c0 + csz],
                                start=(ko == 0),
                                stop=(ko == KO - 1),
                            )
                    for ci, (c0, csz) in enumerate(o_chunks):
                        nc.scalar.copy(ot[:, c0 : c0 + csz], pss[ci])
                    nc.sync.dma_start(out[t0 : t0 + P, :], ot)

    nc.compile()
    return nc


def _run(nc, in_maps, trace=False):
    global LAST_RESULT
    res = run_bass_kernel_spmd(
        nc, in_maps, core_ids=list(range(len(in_maps))), trace=trace
    )
    LAST_RESULT = res
    return res


def kernel(x, weight):
    T = FULL_B * FULL_S
    K = FULL_K
    OS = FULL_O // N_CORES  # 1376
    x2d = np.ascontiguousarray(np.asarray(x, dtype=np.float32).reshape(T, K).T)
    w = np.asarray(weight, dtype=np.float32)

    nc = build_program(K, T, OS)
    in_maps = [
        {"xt": x2d, "w": np.ascontiguousarray(w[c * OS : (c + 1) * OS])}
        for c in range(N_CORES)
    ]
    trace = bool(os.environ.get("BASS_TRACE"))
    res = _run(nc, in_maps, trace=trace)
    full = np.concatenate(
        [res.results[c]["out"] for c in range(N_CORES)], axis=1
    )
    return np.ascontiguousarray(full.reshape(FULL_B, FULL_S, FULL_O))
